# revision 8
# baseline (speedup 1.0000x reference)
"""MoE gate (top-2 of 64 experts) Trainium2 Bass kernel.

Problem: hidden_states [4, 4096, 2048] f32, gate weight [64, 2048] f32.
  logits = x @ W.T            [16384, 64]
  scores = softmax(logits)
  topk_w, topk_i = top_k(scores, 2); topk_w normalized by their sum
  aux_loss from per-batch expert counts (ce) and mean scores.

Sharding: data-parallel over batch*seq. 16384 rows -> 2048 rows/core on 8
cores; the [64, 2048] gate weight is replicated (passed pre-transposed).
x is passed per-core pre-transposed ([D, rows] layout) so the contraction
dim D lands on SBUF partitions with fully-contiguous DMA lines.

Per core device program (Tile framework):
  4 stages x 512 rows. Per stage: 16 k-chunk DMAs [128, 512] feed
  fp32 matmuls accumulating logits into one PSUM tile [128, 4*64]
  (4 row-tiles of 128 rows side by side). Stats: DVE max8/max_index give
  top-2 values+indices per row; ACT exp(logits - max) with accum_out gives
  e and its row-sum; softmax column-sums for the aux loss accumulate on the
  PE via a [128,1] x [128,64] matmul with 1/rowsum as the stationary.
  ce and the final scalar aux loss are reduced on host from returned
  per-core indices and score sums (tiny [8,64] + [16384,2] tensors).
"""

import numpy as np
from contextlib import ExitStack

import concourse.bass as bass
import concourse.tile as tile
from concourse import bacc, mybir
from concourse.bass_utils import run_bass_kernel_spmd

# problem constants (hardcoded per harness contract)
B, S, D, E = 4, 4096, 2048, 64
TOP_K = 2
ALPHA = 0.01
N_CORES = 8
R = (B * S) // N_CORES  # 2048 rows per core
KC = D // 128           # 16 contraction chunks
STAGES = 4
SR = R // STAGES        # 512 rows per stage
JT = SR // 128          # 4 row-tiles per stage

F32 = mybir.dt.float32
U32 = mybir.dt.uint32


def build_moe_gate_kernel():
    nc = bacc.Bacc("TRN2", target_bir_lowering=False, debug=False)

    xT = nc.dram_tensor("xT", [D, R], F32, kind="ExternalInput").ap()
    wT = nc.dram_tensor("wT", [D, E], F32, kind="ExternalInput").ap()
    idx_out = nc.dram_tensor("idx_out", [R, TOP_K], U32, kind="ExternalOutput").ap()
    w_out = nc.dram_tensor("w_out", [R, TOP_K], F32, kind="ExternalOutput").ap()
    ssum_out = nc.dram_tensor("ssum_out", [1, E], F32, kind="ExternalOutput").ap()

    with tile.TileContext(nc) as tc, ExitStack() as ctx:
        wpool = ctx.enter_context(tc.tile_pool(name="w", bufs=1))
        xpool = ctx.enter_context(tc.tile_pool(name="x", bufs=4))
        # one PSUM bank per accumulation group (zero-region granularity = bank);
        # 7 rolling logit banks + 1 ssum bank = all 8 banks
        lgpool = ctx.enter_context(tc.tile_pool(name="lg", bufs=7, space="PSUM"))
        sspool = ctx.enter_context(tc.tile_pool(name="ss", bufs=1, space="PSUM"))
        spool = ctx.enter_context(tc.tile_pool(name="st", bufs=2))
        epool = ctx.enter_context(tc.tile_pool(name="e", bufs=2 * JT))

        # gate weight, all 16 chunks side by side: chunk k at [:, k*E:(k+1)*E]
        wt_sb = wpool.tile([128, KC * E], F32)
        nc.sync.dma_start(wt_sb[:], wT.rearrange("(k p) e -> p k e", p=128))

        # whole-kernel softmax column-sum accumulator; allocated full-height
        # so the bank isn't shared with (and zeroed under) the logits tiles
        ssum_t = sspool.tile([128, E], F32)
        ssum = ssum_t[0:1, :]

        for s in range(STAGES):
            lg = [lgpool.tile([128, E], F32, tag="lg", name=f"lg_s{s}j{j}") for j in range(JT)]
            for k in range(KC):
                xk = xpool.tile([128, SR], F32)
                nc.sync.dma_start(
                    xk[:], xT[k * 128 : (k + 1) * 128, s * SR : (s + 1) * SR]
                )
                for j in range(JT):
                    nc.tensor.matmul(
                        lg[j][:],
                        lhsT=xk[:, j * 128 : (j + 1) * 128],
                        rhs=wt_sb[:, k * E : (k + 1) * E],
                        start=(k == 0),
                        stop=(k == KC - 1),
                    )

            # copy logits out of PSUM once; all stats read SBUF
            lgs = spool.tile([128, JT * E], F32)
            for j in range(JT):
                nc.vector.tensor_copy(lgs[:, j * E : (j + 1) * E], lg[j][:])

            vals8 = spool.tile([128, JT * 8], F32)
            idx8 = spool.tile([128, JT * 8], U32)
            for j in range(JT):
                nc.vector.max(vals8[:, j * 8 : (j + 1) * 8], lgs[:, j * E : (j + 1) * E])
                nc.vector.max_index(
                    idx8[:, j * 8 : (j + 1) * 8],
                    vals8[:, j * 8 : (j + 1) * 8],
                    lgs[:, j * E : (j + 1) * E],
                )

            v3 = vals8.rearrange("p (j c) -> p j c", c=8)
            l1 = v3[:, :, 0:1]  # [128, JT, 1] top-1 logit
            l2 = v3[:, :, 1:2]  # top-2 logit

            negm1 = spool.tile([128, JT], F32)
            nc.vector.tensor_scalar_mul(negm1[:], l1, -1.0)
            d21 = spool.tile([128, JT], F32)
            nc.vector.tensor_sub(d21[:], l2, l1)

            denom = spool.tile([128, JT], F32)
            ev = []
            for j in range(JT):
                e_j = epool.tile([128, E], F32, tag="ev")
                nc.scalar.activation(
                    e_j[:],
                    lgs[:, j * E : (j + 1) * E],
                    mybir.ActivationFunctionType.Exp,
                    bias=negm1[:, j : j + 1],
                    scale=1.0,
                    accum_out=denom[:, j : j + 1],
                )
                ev.append(e_j)

            recip = spool.tile([128, JT], F32)
            nc.vector.reciprocal(recip[:], denom[:])

            # e2 = exp(l2 - l1); w1 = 1/(1+e2); w2 = e2 * w1
            e2 = spool.tile([128, JT], F32)
            nc.scalar.activation(e2[:], d21[:], mybir.ActivationFunctionType.Exp)
            den2 = spool.tile([128, JT], F32)
            nc.vector.tensor_scalar_add(den2[:], e2[:], 1.0)
            wq = spool.tile([128, JT * 2], F32)
            wq3 = wq.rearrange("p (j c) -> p j c", c=2)
            nc.vector.reciprocal(wq3[:, :, 0:1], den2[:])
            nc.vector.tensor_mul(wq3[:, :, 1:2], e2[:], wq3[:, :, 0:1])

            # scores column-sum: ssum += recip_j^T @ e_j  (over all stages)
            for j in range(JT):
                nc.tensor.matmul(
                    ssum[:],
                    lhsT=recip[:, j : j + 1],
                    rhs=ev[j][:],
                    start=(s == 0 and j == 0),
                    stop=(s == STAGES - 1 and j == JT - 1),
                )

            # outputs: rows s*SR + j*128 + p, cols c
            i3 = idx8.rearrange("p (j c) -> p j c", c=8)
            dst_i = idx_out[s * SR : (s + 1) * SR, :].rearrange(
                "(j p) c -> p j c", p=128
            )
            nc.sync.dma_start(dst_i, i3[:, :, 0:TOP_K])
            dst_w = w_out[s * SR : (s + 1) * SR, :].rearrange(
                "(j p) c -> p j c", p=128
            )
            nc.sync.dma_start(dst_w, wq3[:, :, :])

        # final: score sums to DRAM (via SBUF; DMA from PSUM is restricted)
        ssum_sb = spool.tile([1, E], F32)
        nc.vector.tensor_copy(ssum_sb[:], ssum[:])
        nc.sync.dma_start(ssum_out, ssum_sb[:])

    nc.compile()
    return nc


_NC_CACHE = None

# test-harness knobs (harness never touches these; kernel() defaults are fine)
TRACE = False
TMPDIR = None
LAST_RESULT = None


def _get_nc():
    global _NC_CACHE
    if _NC_CACHE is None:
        _NC_CACHE = build_moe_gate_kernel()
    return _NC_CACHE


def kernel(hidden_states: np.ndarray, weight: np.ndarray):
    global LAST_RESULT
    nc = _get_nc()
    x = np.asarray(hidden_states, dtype=np.float32).reshape(B * S, D)
    w = np.asarray(weight, dtype=np.float32)
    wT_np = np.ascontiguousarray(w.T)  # [D, E]

    in_maps = []
    for c in range(N_CORES):
        xT_c = np.ascontiguousarray(x[c * R : (c + 1) * R, :].T)  # [D, R]
        in_maps.append({"xT": xT_c, "wT": wT_np})

    res = run_bass_kernel_spmd(
        nc, in_maps, list(range(N_CORES)), trace=TRACE, tmpdir=TMPDIR
    )
    LAST_RESULT = res
    results = res.results

    idx = np.concatenate([results[c]["idx_out"] for c in range(N_CORES)], axis=0)
    idx = idx.astype(np.int32)  # values 0..63; uint32 -> int32 exact
    tw = np.concatenate([results[c]["w_out"] for c in range(N_CORES)], axis=0)
    ssum = np.stack([results[c]["ssum_out"][0] for c in range(N_CORES)])  # [8, E]

    # host-side tiny reductions for the aux loss
    cores_per_batch = N_CORES // B  # 2
    mean_scores = np.zeros((B, E), np.float32)
    for b in range(B):
        mean_scores[b] = (
            ssum[b * cores_per_batch : (b + 1) * cores_per_batch].sum(axis=0) / S
        )
    idx_b = idx.reshape(B, S * TOP_K)
    ce = np.zeros((B, E), np.float32)
    for b in range(B):
        ce[b] = np.bincount(idx_b[b], minlength=E).astype(np.float32)
    ce /= S * TOP_K / E
    aux_loss = np.float32((ce * mean_scores).sum(axis=1).mean() * ALPHA)

    return idx, tw, aux_loss
